# revision 25
# baseline (speedup 1.0000x reference)
"""Trainium2 Bass kernel for nn_CustomTransformerDecoder.

Sharding: data-parallel over batch for the 4 transformer layers (8 cores x 1
batch element each), then a second launch with the vocab projection
column-sharded over the 8 cores (each core reads only its 1/8 slice of W2).

Per-core transformer layout choices:
  - h kept as [128, 8(s_tile), 512(e)] in SBUF (s on partitions).
  - LN stats per partition row (free-dim reduce), xn built by one ACT op
    (scale=rstd, bias=-mu*rstd per partition).
  - xn transposed (PE) to xnT [e, s] which feeds all QKV/FF matmuls.
  - qT/kT produced transposed ([hd, s]) so attention scores are computed
    transposed: scoresT[sk, sq] = kT_h.T @ qT_h (K = D = 64).
  - softmax without max subtraction (scores are ~±0.3 here), causal handled
    by only computing lower-triangular (sq >= sk) blocks; diagonal blocks
    masked post-exp with gpsimd.affine_select. Upper blocks of the attention
    map outputs are never written (output buffers are pre-zeroed).
  - ctx = expT.T @ [v | 1]: the appended ones column yields the softmax row
    sums for free; normalization folded into the PSUM eviction.
  - attention map (head 7) blocks PE-transposed back to [sq, sk], scaled by
    the row reciprocal, and DMA'd out row-block by row-block.
"""

import os
from functools import lru_cache

import numpy as np

import concourse.bacc as bacc
import concourse.bass as bass
import concourse.mybir as mybir
from concourse import bass_utils
from concourse.bass import IndirectOffsetOnAxis
from concourse.masks import make_identity
from concourse.tile import TileContext

B, S, E, H, D, L, NH, V = 8, 1024, 512, 8, 64, 4, 2048, 32000
P = 128
NCORES = 8
VS = V // NCORES  # vocab shard per core
HD = H * D  # 512
NT = S // P  # 8 s tiles
KE = E // P  # 4 e tiles
FP = mybir.dt.float32
BF = mybir.dt.bfloat16
SCALE = float(S) ** -0.5

AluOp = mybir.AluOpType
_last_flags1 = (False, False, False, False)
_last_flags2 = (False, False)
Act = mybir.ActivationFunctionType


def _pos_encoding():
    pos = np.arange(S, dtype=np.float32)[:, None]
    div = np.exp(
        np.arange(0, E, 2, dtype=np.float32) * (-np.log(10000.0) / E)
    ).astype(np.float32)
    pe = np.zeros((S, E), dtype=np.float32)
    pe[:, 0::2] = np.sin(pos * div)
    pe[:, 1::2] = np.cos(pos * div)
    return pe


# ---------------------------------------------------------------------------
# Phase 1: embedding + 4 transformer layers (one batch element per core)
# ---------------------------------------------------------------------------


@lru_cache(maxsize=None)
def build_phase1(with_lngb=False, with_qkv_bias=False, with_bff=False,
                 debug_h=False):
    nc = bacc.Bacc("TRN2", target_bir_lowering=False, debug=False,
                   enable_asserts=False)

    xi_d = nc.dram_tensor("xi", [NT, P, 1], mybir.dt.int32,
                          kind="ExternalInput").ap()
    emb_d = nc.dram_tensor("emb", [V, E], FP, kind="ExternalInput").ap()
    pe_d = nc.dram_tensor("pe", [S, E], FP, kind="ExternalInput").ap()
    wq_d = nc.dram_tensor("wq", [L, E, HD], BF, kind="ExternalInput").ap()
    wk_d = nc.dram_tensor("wk", [L, E, HD], BF, kind="ExternalInput").ap()
    wv_d = nc.dram_tensor("wv", [L, E, HD], BF, kind="ExternalInput").ap()
    wff_d = nc.dram_tensor("wff", [L, E, E], BF, kind="ExternalInput").ap()
    if with_lngb:
        # pre-broadcast across partitions host-side
        lng_d = nc.dram_tensor("lng", [L, P, E], FP, kind="ExternalInput").ap()
        lnb_d = nc.dram_tensor("lnb", [L, P, E], FP, kind="ExternalInput").ap()
    if with_qkv_bias:
        # [L, 4, 128, 1] per-partition layout for qT/kT evictions (bq pre-scaled
        # by SCALE host-side); bv pre-broadcast [L, P, HD] for v (free-dim).
        bq_d = nc.dram_tensor("bqp", [L, KE, P, 1], FP, kind="ExternalInput").ap()
        bk_d = nc.dram_tensor("bkp", [L, KE, P, 1], FP, kind="ExternalInput").ap()
        bv_d = nc.dram_tensor("bvb", [L, P, HD], FP, kind="ExternalInput").ap()
    if with_bff:
        bff_d = nc.dram_tensor("bffb", [L, P, E], FP, kind="ExternalInput").ap()

    maps_d = nc.dram_tensor("maps", [L, S, S], FP, kind="ExternalOutput").ap()
    pooled_d = nc.dram_tensor("pooled", [1, E], FP, kind="ExternalOutput").ap()
    if debug_h:
        hdbg_d = nc.dram_tensor("hdbg", [L, P, NT, E], FP,
                                kind="ExternalOutput").ap()
        xndbg_d = nc.dram_tensor("xndbg", [L, P, NT, E], BF,
                                 kind="ExternalOutput").ap()

    from contextlib import ExitStack
    with TileContext(nc) as tc, ExitStack() as es:
        consts = es.enter_context(tc.tile_pool(name="consts", bufs=1))
        hpool = es.enter_context(tc.tile_pool(name="hbuf", bufs=1))
        wpool = es.enter_context(tc.tile_pool(name="wts", bufs=2))
        spool = es.enter_context(tc.tile_pool(name="scratch", bufs=3))
        tpool = es.enter_context(tc.tile_pool(name="tiles", bufs=2))
        expool = es.enter_context(tc.tile_pool(name="expt", bufs=10))
        stash = es.enter_context(tc.tile_pool(name="stash", bufs=12))
        statp = es.enter_context(tc.tile_pool(name="stats", bufs=2))
        rowp = es.enter_context(tc.tile_pool(name="rows", bufs=2))
        pa = es.enter_context(tc.tile_pool(name="psA", bufs=2, space="PSUM"))
        ps_s = es.enter_context(tc.tile_pool(name="psS", bufs=2, space="PSUM"))
        pctx = es.enter_context(tc.tile_pool(name="psCtx", bufs=2, space="PSUM"))

        ident = consts.tile([P, P], FP)
        make_identity(nc, ident)
        identB = consts.tile([P, P], BF)
        make_identity(nc, identB)
        maskT = consts.tile([P, P], BF)
        nc.vector.memset(maskT, 0.0)
        nc.gpsimd.affine_select(
            out=maskT, in_=maskT, compare_op=AluOp.is_ge, fill=-30000.0,
            base=0, pattern=[[1, P]], channel_multiplier=-1)
        ones_col = consts.tile([P, 1], FP)
        nc.vector.memset(ones_col, 1.0)
        eps_t = consts.tile([P, 1], FP)
        nc.vector.memset(eps_t, 1e-5)

        # ---- embedding gather + positional encoding -> h [128, NT, 512]
        h_sb = hpool.tile([P, NT, E], FP)
        xi_sb = consts.tile([P, NT], mybir.dt.int32)
        nc.sync.dma_start(out=xi_sb, in_=xi_d.rearrange("t p o -> p (t o)"))
        for t in range(NT):
            nc.gpsimd.indirect_dma_start(
                out=h_sb[:, t, :],
                out_offset=None,
                in_=emb_d[:, :],
                in_offset=IndirectOffsetOnAxis(ap=xi_sb[:, t:t + 1], axis=0),
            )
        for t in range(NT):
            pe_t = spool.tile([P, E], FP, tag="xn", name=f"pe_{t}")
            nc.sync.dma_start(out=pe_t, in_=pe_d[t * P:(t + 1) * P, :])
            nc.vector.tensor_add(h_sb[:, t, :], h_sb[:, t, :], pe_t)

        # persistent v buffer (ones columns written once)
        v_sb = hpool.tile([P, NT, H, D + 1], BF)
        nc.vector.memset(v_sb[:, :, :, D:D + 1], 1.0)

        for l in range(L):
            # ---- load weights for this layer
            wq_sb = wpool.tile([P, KE, HD], BF, tag="wq")
            wk_sb = wpool.tile([P, KE, HD], BF, tag="wk")
            wv_sb = wpool.tile([P, KE, HD], BF, tag="wv")
            wff_sb = wpool.tile([P, KE, E], BF, tag="wff")
            nc.sync.dma_start(out=wq_sb, in_=wq_d[l].rearrange("(k p) n -> p k n", p=P))
            nc.sync.dma_start(out=wk_sb, in_=wk_d[l].rearrange("(k p) n -> p k n", p=P))
            nc.sync.dma_start(out=wv_sb, in_=wv_d[l].rearrange("(k p) n -> p k n", p=P))
            nc.sync.dma_start(out=wff_sb, in_=wff_d[l].rearrange("(k p) n -> p k n", p=P))
            if with_lngb:
                lng_sb = wpool.tile([P, E], FP, tag="lng")
                lnb_sb = wpool.tile([P, E], FP, tag="lnb")
                nc.sync.dma_start(out=lng_sb, in_=lng_d[l])
                nc.sync.dma_start(out=lnb_sb, in_=lnb_d[l])
            if with_qkv_bias:
                bq_sb = wpool.tile([P, KE], FP, tag="bq")
                bk_sb = wpool.tile([P, KE], FP, tag="bk")
                bv_sb = wpool.tile([P, HD], FP, tag="bv")
                nc.sync.dma_start(out=bq_sb, in_=bq_d[l].rearrange("k p o -> p (k o)"))
                nc.sync.dma_start(out=bk_sb, in_=bk_d[l].rearrange("k p o -> p (k o)"))
                nc.sync.dma_start(out=bv_sb, in_=bv_d[l])
            if with_bff:
                bff_sb = wpool.tile([P, E], FP, tag="bff")
                nc.sync.dma_start(out=bff_sb, in_=bff_d[l])

            # ---- layernorm stats
            negmu = statp.tile([P, NT], FP, tag="negmu")
            sumsq = statp.tile([P, NT], FP, tag="sumsq")
            rstd = statp.tile([P, NT], FP, tag="rstd")
            negmusc = statp.tile([P, NT], FP, tag="negmusc")
            sq_scr = spool.tile([P, E], FP, tag="sqscr")
            for t in range(NT):
                nc.vector.tensor_reduce(
                    out=negmu[:, t:t + 1], in_=h_sb[:, t, :],
                    axis=mybir.AxisListType.X, op=AluOp.add, negate=True)
                nc.scalar.activation(out=sq_scr, in_=h_sb[:, t, :],
                                     func=Act.Square,
                                     accum_out=sumsq[:, t:t + 1])
            nc.vector.tensor_scalar_mul(negmu, negmu, 1.0 / E)
            # var = sumsq/E - mu^2  (negmu^2 == mu^2)
            var8 = statp.tile([P, NT], FP, tag="var8")
            nc.vector.tensor_mul(var8, negmu, negmu)
            nc.vector.tensor_scalar(out=sumsq, in0=sumsq, scalar1=1.0 / E,
                                    scalar2=None, op0=AluOp.mult)
            nc.vector.tensor_sub(var8, sumsq, var8)
            nc.scalar.activation(out=rstd, in_=var8, func=Act.Sqrt, bias=eps_t)
            nc.vector.reciprocal(rstd, rstd)
            nc.vector.tensor_mul(negmusc, negmu, rstd)

            # ---- xn per tile -> transpose -> xnT [128, KE, S]
            xnT = tpool.tile([P, KE, S], BF, tag="xT")
            for t in range(NT):
                xn_t = spool.tile([P, E], BF, tag="xn")
                nc.scalar.activation(out=xn_t, in_=h_sb[:, t, :],
                                     func=Act.Identity,
                                     scale=rstd[:, t:t + 1],
                                     bias=negmusc[:, t:t + 1])
                if with_lngb:
                    nc.vector.tensor_mul(xn_t, xn_t, lng_sb)
                    nc.vector.tensor_add(xn_t, xn_t, lnb_sb)
                if debug_h:
                    nc.sync.dma_start(out=xndbg_d[l, :, t, :], in_=xn_t)
                tp = pa.tile([P, E], BF, tag="psA", name=f"tpx_{l}_{t}")
                for k in range(KE):
                    nc.tensor.transpose(tp[:, k * P:(k + 1) * P],
                                        xn_t[:, k * P:(k + 1) * P], identB)
                nc.vector.tensor_copy(
                    xnT[:, :, t * P:(t + 1) * P],
                    tp.rearrange("p (k q) -> p k q", q=P))

            # ---- qT, kT [128, KE(m), S]; v [s, hd] into v_sb
            qT = tpool.tile([P, KE, S], BF, tag="qT")
            kT = tpool.tile([P, KE, S], BF, tag="kT")
            for m in range(KE):
                for n in range(2):
                    nsl = slice(n * 512, (n + 1) * 512)
                    pq = pa.tile([P, 512], FP, tag="psA")
                    for k in range(KE):
                        nc.tensor.matmul(
                            pq, wq_sb[:, k, m * P:(m + 1) * P], xnT[:, k, nsl],
                            start=(k == 0), stop=(k == KE - 1))
                    if with_qkv_bias:
                        nc.scalar.activation(
                            out=qT[:, m, nsl], in_=pq, func=Act.Identity,
                            scale=SCALE, bias=bq_sb[:, m:m + 1])
                    else:
                        nc.vector.tensor_scalar_mul(qT[:, m, nsl], pq, SCALE)
                    pk = pa.tile([P, 512], FP, tag="psA")
                    for k in range(KE):
                        nc.tensor.matmul(
                            pk, wk_sb[:, k, m * P:(m + 1) * P], xnT[:, k, nsl],
                            start=(k == 0), stop=(k == KE - 1))
                    if with_qkv_bias:
                        nc.scalar.activation(out=kT[:, m, nsl], in_=pk,
                                             func=Act.Identity,
                                             bias=bk_sb[:, m:m + 1])
                    else:
                        nc.vector.tensor_copy(kT[:, m, nsl], pk)
            for t in range(NT):
                pv = pa.tile([P, 512], FP, tag="psA")
                for k in range(KE):
                    nc.tensor.matmul(pv, xnT[:, k, t * P:(t + 1) * P],
                                     wv_sb[:, k, :],
                                     start=(k == 0), stop=(k == KE - 1))
                nc.vector.tensor_copy(
                    v_sb[:, t, :, :D],
                    pv.rearrange("p (h d) -> p h d", d=D))
                if with_qkv_bias:
                    nc.vector.tensor_add(
                        v_sb[:, t, :, :D], v_sb[:, t, :, :D],
                        bv_sb.rearrange("p (h d) -> p h d", d=D))

            # ---- attention, head pairs
            rec7 = statp.tile([P, NT], FP, tag="rec7")
            map_work = []
            for hp in range(4):
                for half in range(2):
                    i0, i1 = half * 4, half * 4 + 4
                    cps = [pctx.tile([P, 4, D + 1], FP, tag="ctx",
                                     name=f"cps_{l}_{hp}_{half}_{hh}")
                           for hh in range(2)]
                    ex_tiles = {}
                    ex_all = {}
                    for j in range(i1):
                        i_lo = max(j, i0)
                        n0 = i_lo * P
                        ncols = i1 * P - n0
                        diag = j >= i0
                        dcol = j * P - n0
                        sc = ps_s.tile([P, 2, 512], FP, tag="psS",
                                       name=f"sc_{l}_{hp}_{half}_{j}")
                        for hh in range(2):
                            prow = slice(hh * D, (hh + 1) * D)
                            nc.tensor.matmul(
                                sc[:, hh, :ncols],
                                kT[prow, hp, j * P:(j + 1) * P],
                                qT[prow, hp, n0:i1 * P],
                                start=True, stop=not diag)
                            if diag:
                                nc.tensor.matmul(
                                    sc[:, hh, dcol:dcol + P], identB, maskT,
                                    start=False, stop=True)
                        keep = hp == 3
                        pool_ = stash if keep else expool
                        ex = pool_.tile([P, 2, 512], BF,
                                        tag=("stash" if keep else "ex"),
                                        name=f"ex_{l}_{hp}_{half}_{j}")
                        nc.scalar.activation(out=ex[:, :, :ncols],
                                             in_=sc[:, :, :ncols], func=Act.Exp)
                        ex_all[j] = (ex, n0)
                        if keep:
                            ex_tiles[j] = (ex, n0)
                    for j in range(i1):
                        i_lo = max(j, i0)
                        ex, n0 = ex_all[j]
                        for hh in range(2):
                            h_idx = 2 * hp + hh
                            for i in range(i_lo, i1):
                                col = i * P - n0
                                nc.tensor.matmul(
                                    cps[hh][:, i - i0, :],
                                    ex[:, hh, col:col + P],
                                    v_sb[:, j, h_idx, :],
                                    start=(j == 0 and i == i_lo),
                                    stop=(j == i1 - 1 and i == i1 - 1))
                    # evict ctx into h (+ stash head-7 recips)
                    rec4 = statp.tile([P, 2, 4], FP, tag="rec")
                    ctxs = spool.tile([P, 2, 4, D], FP, tag="ctxs")
                    for hh in range(2):
                        nc.vector.reciprocal(rec4[:, hh, :], cps[hh][:, :, D])
                        nc.vector.tensor_mul(
                            ctxs[:, hh], cps[hh][:, :, :D],
                            rec4[:, hh][:].unsqueeze(2).to_broadcast([P, 4, D]))
                    if hp == 3:
                        nc.vector.tensor_copy(rec7[:, i0:i1], rec4[:, 1, :])
                    for i in range(i0, i1):
                        hs = h_sb[:, i, hp * P:(hp + 1) * P].rearrange(
                            "p (h d) -> p h d", d=D)
                        nc.gpsimd.tensor_add(hs, hs, ctxs[:, :, i - i0, :])
                    if hp == 3:
                        map_work.append((i0, i1, ex_tiles))

            # ---- h -> hT, FF, residuals
            hT = tpool.tile([P, KE, S], BF, tag="xT")
            for t in range(NT):
                tp = pa.tile([P, E], FP, tag="psA")
                for k in range(KE):
                    nc.tensor.transpose(tp[:, k * P:(k + 1) * P],
                                        h_sb[:, t, k * P:(k + 1) * P], ident)
                nc.vector.tensor_copy(
                    hT[:, :, t * P:(t + 1) * P],
                    tp.rearrange("p (k q) -> p k q", q=P))
            for t in range(NT):
                pf = pa.tile([P, E], FP, tag="psA")
                for k in range(KE):
                    nc.tensor.matmul(pf, hT[:, k, t * P:(t + 1) * P],
                                     wff_sb[:, k, :],
                                     start=(k == 0), stop=(k == KE - 1))
                nc.vector.tensor_add(h_sb[:, t, :], h_sb[:, t, :], pf)
                if with_bff:
                    nc.vector.tensor_add(h_sb[:, t, :], h_sb[:, t, :], bff_sb)
                if debug_h:
                    nc.sync.dma_start(out=hdbg_d[l, :, t, :], in_=h_sb[:, t, :])

            # ---- map output (head 7), fills gaps alongside FF / next-layer LN
            for (i0, i1, ex_tiles) in map_work:
                for i in range(i0, i1):
                    row = rowp.tile([P, S], FP, tag="row",
                                    name=f"row_{l}_{i}")
                    for j in range(i + 1):
                        ex, n0 = ex_tiles[j]
                        mp = pa.tile([P, P], BF, tag="psA",
                                     name=f"mp_{l}_{i}_{j}")
                        nc.tensor.transpose(
                            mp, ex[:, 1, i * P - n0:i * P - n0 + P], identB)
                        nc.vector.tensor_scalar_mul(
                            row[:, j * P:(j + 1) * P], mp,
                            rec7[:, i:i + 1])
                    nc.sync.dma_start(
                        out=maps_d[l, i * P:(i + 1) * P, :(i + 1) * P],
                        in_=row[:, :(i + 1) * P])

        # ---- pooled = sum over s
        pp = pa.tile([1, E], FP, tag="psA")
        for t in range(NT):
            nc.tensor.matmul(pp, ones_col, h_sb[:, t, :],
                             start=(t == 0), stop=(t == NT - 1))
        pooled_sb = statp.tile([1, E], FP, tag="pooled")
        nc.vector.tensor_copy(pooled_sb, pp)
        nc.sync.dma_start(out=pooled_d, in_=pooled_sb)

    nc.compile()
    return nc


# ---------------------------------------------------------------------------
# Phase 2: vocab projection, column-sharded
# ---------------------------------------------------------------------------


@lru_cache(maxsize=None)
def build_phase2(with_b1=False, with_b2=False):
    nc = bacc.Bacc("TRN2", target_bir_lowering=False, debug=False,
                   enable_asserts=False)
    KF = NH // P  # 16
    NCH = 8  # vocab chunks per core
    CW = VS // NCH  # 500

    pt_d = nc.dram_tensor("pooledT", [E, B], FP, kind="ExternalInput").ap()
    w1_d = nc.dram_tensor("w1", [E, NH], FP, kind="ExternalInput").ap()
    w2_d = nc.dram_tensor("w2s", [NH, VS], BF, kind="ExternalInput").ap()
    if with_b1:
        b1_d = nc.dram_tensor("b1b", [B, NH], FP, kind="ExternalInput").ap()
    if with_b2:
        b2_d = nc.dram_tensor("b2b", [B, VS], FP, kind="ExternalInput").ap()
    lg_d = nc.dram_tensor("logits", [B, VS], FP, kind="ExternalOutput").ap()

    from contextlib import ExitStack
    with TileContext(nc) as tc, ExitStack() as es:
        consts = es.enter_context(tc.tile_pool(name="consts", bufs=1))
        sp = es.enter_context(tc.tile_pool(name="sb", bufs=2))
        w2p = es.enter_context(tc.tile_pool(name="w2", bufs=16))
        pa = es.enter_context(tc.tile_pool(name="psA", bufs=3, space="PSUM"))
        pl_p = es.enter_context(tc.tile_pool(name="psL", bufs=2, space="PSUM"))

        ident = consts.tile([P, P], FP)
        make_identity(nc, ident)
        identB = consts.tile([P, P], BF)
        make_identity(nc, identB)
        maskT = consts.tile([P, P], BF)
        nc.vector.memset(maskT, 0.0)
        nc.gpsimd.affine_select(
            out=maskT, in_=maskT, compare_op=AluOp.is_ge, fill=-30000.0,
            base=0, pattern=[[1, P]], channel_multiplier=-1)

        pt_sb = consts.tile([P, KE, B], FP)
        nc.sync.dma_start(out=pt_sb, in_=pt_d.rearrange("(k p) b -> p k b", p=P))
        w1_sb = consts.tile([P, KE, NH], FP)
        nc.sync.dma_start(out=w1_sb, in_=w1_d.rearrange("(k p) n -> p k n", p=P))
        if with_b1:
            b1_sb = consts.tile([B, NH], FP)
            nc.sync.dma_start(out=b1_sb, in_=b1_d)

        hid = sp.tile([B, NH], FP, tag="hid", bufs=1)
        for n in range(KF // 4):  # 4 chunks of 512
            ph = pa.tile([B, 512], FP, tag="psA")
            for k in range(KE):
                nc.tensor.matmul(ph, pt_sb[:, k, :],
                                 w1_sb[:, k, n * 512:(n + 1) * 512],
                                 start=(k == 0), stop=(k == KE - 1))
            if with_b1:
                nc.vector.tensor_add(hid[:, n * 512:(n + 1) * 512],
                                     ph, b1_sb[:, n * 512:(n + 1) * 512])
                nc.vector.tensor_scalar_max(hid[:, n * 512:(n + 1) * 512],
                                            hid[:, n * 512:(n + 1) * 512], 0.0)
            else:
                nc.scalar.activation(out=hid[:, n * 512:(n + 1) * 512],
                                     in_=ph, func=Act.Relu)

        # transpose hidden -> hidT [128, KF, B]
        hidT = sp.tile([P, KF, B], BF, tag="hidT")
        pt2 = pa.tile([P, KF * B], FP, tag="psA")
        for k2 in range(KF):
            nc.tensor.transpose(pt2[:, k2 * B:(k2 + 1) * B],
                                hid[:, k2 * P:(k2 + 1) * P], ident[:B, :B])
        nc.vector.tensor_copy(hidT, pt2.rearrange("p (k b) -> p k b", b=B))

        lg_sb = sp.tile([B, VS], FP, tag="lg", bufs=1)
        w2rows = []
        for k2 in range(KF):
            w2t = w2p.tile([P, VS], BF, tag="w2t", name=f"w2row_{k2}")
            nc.sync.dma_start(out=w2t, in_=w2_d[k2 * P:(k2 + 1) * P, :])
            w2rows.append(w2t)
        for n in range(NCH):
            pl = pl_p.tile([B, CW], FP, tag="psL")
            for k2 in range(KF):
                nc.tensor.matmul(pl, hidT[:, k2, :],
                                 w2rows[k2][:, n * CW:(n + 1) * CW],
                                 start=(k2 == 0), stop=(k2 == KF - 1))
            if with_b2:
                b2c = sp.tile([B, CW], FP, tag="b2c", name=f"b2c_{n}")
                nc.sync.dma_start(out=b2c, in_=b2_d[:, n * CW:(n + 1) * CW])
                nc.vector.tensor_add(lg_sb[:, n * CW:(n + 1) * CW], pl, b2c)
            else:
                nc.vector.tensor_copy(lg_sb[:, n * CW:(n + 1) * CW], pl)
        nc.sync.dma_start(out=lg_d, in_=lg_sb)

    nc.compile()
    return nc


# ---------------------------------------------------------------------------
# Host glue
# ---------------------------------------------------------------------------


def _prep(a):
    return np.ascontiguousarray(np.asarray(a, dtype=np.float32))


def _bf16(a):
    import ml_dtypes
    return np.ascontiguousarray(np.asarray(a).astype(ml_dtypes.bfloat16))


def kernel(x, emb, ln_g, ln_b, Wq, bq, Wk, bk, Wv, bv, Wff, bff, W1, b1, W2, b2):
    x = np.asarray(x)
    x_i = np.ascontiguousarray(x.astype(np.int32))
    emb = _prep(emb)
    ln_g, ln_b = _prep(ln_g), _prep(ln_b)
    Wq, bq, Wk, bk = _prep(Wq), _prep(bq), _prep(Wk), _prep(bk)
    Wv, bv, Wff, bff = _prep(Wv), _prep(bv), _prep(Wff), _prep(bff)
    W1, b1, W2, b2 = _prep(W1), _prep(b1), _prep(W2), _prep(b2)

    with_lngb = not (np.all(ln_g == 1.0) and np.all(ln_b == 0.0))
    with_qkv_bias = not (np.all(bq == 0) and np.all(bk == 0) and np.all(bv == 0))
    with_bff = not np.all(bff == 0)
    with_b1 = not np.all(b1 == 0)
    with_b2 = not np.all(b2 == 0)

    pe = _pos_encoding()
    # [L, H, E, D] -> [L, E, H*D]
    wq_all = np.ascontiguousarray(Wq.transpose(0, 2, 1, 3).reshape(L, E, HD))
    wk_all = np.ascontiguousarray(Wk.transpose(0, 2, 1, 3).reshape(L, E, HD))
    wv_all = np.ascontiguousarray(Wv.transpose(0, 2, 1, 3).reshape(L, E, HD))
    Wff = np.ascontiguousarray(Wff)

    trace = bool(int(os.environ.get("BASS_KERNEL_TRACE", "0")))

    global _last_flags1, _last_flags2
    _last_flags1 = (with_lngb, with_qkv_bias, with_bff, False)
    _last_flags2 = (with_b1, with_b2)

    nc1 = build_phase1(with_lngb, with_qkv_bias, with_bff, False)
    in_maps1 = []
    for c in range(NCORES):
        m = {
            "xi": np.ascontiguousarray(x_i[c].reshape(NT, P, 1)),
            "emb": emb,
            "pe": pe,
            "wq": _bf16(wq_all),
            "wk": _bf16(wk_all),
            "wv": _bf16(wv_all),
            "wff": _bf16(Wff),
        }
        if with_lngb:
            m["lng"] = np.ascontiguousarray(
                np.broadcast_to(ln_g[:, None, :], (L, P, E)))
            m["lnb"] = np.ascontiguousarray(
                np.broadcast_to(ln_b[:, None, :], (L, P, E)))
        if with_qkv_bias:
            bqf = (bq.reshape(L, HD) * SCALE).reshape(L, KE, P, 1)
            bkf = bk.reshape(L, HD).reshape(L, KE, P, 1)
            m["bqp"] = np.ascontiguousarray(bqf)
            m["bkp"] = np.ascontiguousarray(bkf)
            m["bvb"] = np.ascontiguousarray(
                np.broadcast_to(bv.reshape(L, 1, HD), (L, P, HD)))
        if with_bff:
            m["bffb"] = np.ascontiguousarray(
                np.broadcast_to(bff[:, None, :], (L, P, E)))
        in_maps1.append(m)

    r1 = bass_utils.run_bass_kernel_spmd(nc1, in_maps1,
                                         list(range(NCORES)), trace=trace)
    if trace and r1.exec_time_ns:
        print(f"phase1 exec_time_ns: {r1.exec_time_ns}")

    pooled_all = np.stack([r1.results[c]["pooled"][0] for c in range(NCORES)])
    pooledT = np.ascontiguousarray(pooled_all.T)  # [E, B]

    nc2 = build_phase2(with_b1, with_b2)
    in_maps2 = []
    for c in range(NCORES):
        m = {
            "pooledT": pooledT,
            "w1": W1,
            "w2s": _bf16(W2[:, c * VS:(c + 1) * VS]),
        }
        if with_b1:
            m["b1b"] = np.ascontiguousarray(np.broadcast_to(b1[None, :], (B, NH)))
        if with_b2:
            m["b2b"] = np.ascontiguousarray(
                np.broadcast_to(b2[None, c * VS:(c + 1) * VS], (B, VS)))
        in_maps2.append(m)

    r2 = bass_utils.run_bass_kernel_spmd(nc2, in_maps2,
                                         list(range(NCORES)), trace=trace)
    if trace and r2.exec_time_ns:
        print(f"phase2 exec_time_ns: {r2.exec_time_ns}")

    logits = np.concatenate([r2.results[c]["logits"] for c in range(NCORES)],
                            axis=1)
    maps = [np.stack([r1.results[c]["maps"][l] for c in range(NCORES)])
            for l in range(L)]
    return (logits, *maps)
